# revision 15
# baseline (speedup 1.0000x reference)
import sys

sys.path.insert(0, "/opt/trn_rl_repo")

import numpy as np

B, C, H, W = 8, 81, 96, 320
F = H * W
NBOX = 16
ALPHA = 0.25
DEPTH_MIN, DEPTH_MAX, NUM_BINS = 0.001, 60.0, 80

STRIDE = 32.0
OFF = 16.0
BIG = 1024.0
K = 7
KCOL = W * K

VC = 32
FC = VC * W
HPX = W // 2
PXCOL = (H - VC) * W * C // 128

NCH = 4
CHCOL = FC // NCH
PX_CHUNKS = ((0, 48), (48, 48), (96, 44), (140, 20))

CH_SPLIT = (1824, 608, 128)
PX_GSPLIT = {48: (26, 19, 3), 44: (24, 17, 3), 20: (11, 7, 2)}

A_SCH = 1024.0 * np.float64(np.log2(np.e))
B_SCH = 15360.0 - 60.0
LN2 = float(np.log(2.0))
SLN = 0.0435

_PROG = None


def _build_program():
    from concourse import bacc, tile, mybir

    f32 = mybir.dt.float32
    f16 = mybir.dt.float16
    bf16 = mybir.dt.bfloat16
    i16 = mybir.dt.int16
    i32 = mybir.dt.int32
    f8 = mybir.dt.float8e4
    AF = mybir.ActivationFunctionType
    OP = mybir.AluOpType

    nc = bacc.Bacc(
        "TRN2",
        target_bir_lowering=False,
        debug=False,
        enable_asserts=False,
    )

    lch_d = nc.dram_tensor("lch", [C, FC], f8, kind="ExternalInput")
    lpx_d = nc.dram_tensor("lpx", [128, PXCOL], f8, kind="ExternalInput")
    lgat_d = nc.dram_tensor("lgat", [H, KCOL], f8, kind="ExternalInput")
    bdcw_d = nc.dram_tensor("bdcw", [18, KCOL + H], bf16, kind="ExternalInput")
    consts8_d = nc.dram_tensor("consts8", [H, VC * VC + H], f8, kind="ExternalInput")
    out_d = nc.dram_tensor("out", [1, 1], f32, kind="ExternalOutput")

    import os

    dbg = os.environ.get("KERNEL_DEBUG") == "1"
    if dbg:
        dbg_m = nc.dram_tensor("dbg_m", [H, W], f32, kind="ExternalOutput")

    with tile.TileContext(nc) as tc:
        with (
            tc.tile_pool(name="persist", bufs=1) as pp,
            tc.tile_pool(name="tree", bufs=2) as tp,
            tc.tile_pool(name="spsum", bufs=1, space="PSUM") as sp,
            tc.tile_pool(name="ppsum", bufs=1, space="PSUM") as qp,
            tc.tile_pool(name="opsum", bufs=1, space="PSUM") as op_,
        ):
            lch = pp.tile([C, FC], f8)
            lpx = pp.tile([128, PXCOL], f8)
            ec = pp.tile([C, FC], f16)
            epx = pp.tile([128, PXCOL], f16)
            lgat = pp.tile([H, KCOL], f8)
            bdcw = pp.tile([18, KCOL + H], bf16)
            consts8 = pp.tile([H, VC * VC + H], f8)
            s_px = pp.tile([128, HPX], f32)
            ones96 = pp.tile([H, 1], bf16)
            ones128 = pp.tile([128, 1], bf16)
            nc.vector.memset(ones96[:], 1.0)
            nc.vector.memset(ones128[:], 1.0)
            diag32 = consts8[0:C, 0 : VC * VC]
            ident96 = consts8[:, VC * VC : VC * VC + H]

            def dma_lch(ring, k):
                ring.dma_start(
                    lch[:, k * CHCOL : (k + 1) * CHCOL],
                    lch_d[:, k * CHCOL : (k + 1) * CHCOL],
                )

            def dma_lpx(ring, k):
                g0, gn = PX_CHUNKS[k]
                ring.dma_start(
                    lpx[:, g0 * C : (g0 + gn) * C], lpx_d[:, g0 * C : (g0 + gn) * C]
                )

            dma_lch(nc.sync, 0)
            dma_lpx(nc.sync, 1)
            dma_lch(nc.sync, 3)
            dma_lpx(nc.scalar, 0)
            dma_lch(nc.scalar, 2)
            dma_lpx(nc.scalar, 3)
            nc.gpsimd.dma_start(consts8[:], consts8_d[:])
            nc.gpsimd.dma_start(bdcw[:], bdcw_d[:])
            nc.gpsimd.dma_start(lgat[:], lgat_d[:])
            dma_lch(nc.gpsimd, 1)
            dma_lpx(nc.gpsimd, 2)

            s_ps = sp.tile([VC, W], f32)
            pen = qp.tile([H, KCOL], f32)
            mstar = pp.tile([H, W], f32)

            w18 = bdcw[:, KCOL : KCOL + H]
            for c0 in range(0, KCOL, 512):
                cn = min(512, KCOL - c0)
                nc.tensor.matmul(
                    pen[:, c0 : c0 + cn],
                    w18,
                    bdcw[:, c0 : c0 + cn],
                    start=True,
                    stop=False,
                )

            def exp_split(dst, src, base, widths):
                a, p, d = widths
                c0 = base
                nc.scalar.activation(dst[:, c0 : c0 + a], src[:, c0 : c0 + a], AF.Exp)
                c0 += a
                nc.gpsimd.tensor_scalar(
                    dst[:, c0 : c0 + p].bitcast(i16),
                    src[:, c0 : c0 + p],
                    A_SCH,
                    B_SCH,
                    op0=OP.mult,
                    op1=OP.add,
                )
                c0 += p
                if d:
                    nc.vector.tensor_scalar(
                        dst[:, c0 : c0 + d].bitcast(i16),
                        src[:, c0 : c0 + d],
                        A_SCH,
                        B_SCH,
                        op0=OP.mult,
                        op1=OP.add,
                    )

            rows_per_chunk = VC // NCH
            for k in range(4):
                g0, gn = PX_CHUNKS[k]
                ga, gp, gd = PX_GSPLIT[gn]
                exp_split(epx, lpx, g0 * C, (ga * C, gp * C, gd * C))
                exp_split(ec, lch, k * CHCOL, CH_SPLIT)
                for r in range(rows_per_chunk):
                    v = k * rows_per_chunk + r
                    nc.tensor.matmul(
                        s_ps[:],
                        diag32[:, VC * v : VC * (v + 1)],
                        ec[:, v * W : (v + 1) * W],
                        start=(v == 0),
                        stop=(v == VC - 1),
                    )
                g3 = epx[:, g0 * C : (g0 + gn) * C].rearrange("p (g c) -> p g c", c=C)
                tb = tp.tile([128, gn * 40], f16, tag=f"tb{gn}")
                tc_ = tp.tile([128, gn * 20], f16, tag=f"tc{gn}")
                td = tp.tile([128, gn * 11], f16, tag=f"td{gn}")
                b3 = tb[:].rearrange("p (g c) -> p g c", c=40)
                c3 = tc_[:].rearrange("p (g c) -> p g c", c=20)
                d3 = td[:].rearrange("p (g c) -> p g c", c=11)
                TT = nc.vector.tensor_tensor
                TT(b3, g3[:, :, 0:40], g3[:, :, 40:80], op=OP.add)
                nc.vector.tensor_copy(d3[:, :, 10:11], g3[:, :, 80:81])
                TT(c3, b3[:, :, 0:20], b3[:, :, 20:40], op=OP.add)
                TT(d3[:, :, 0:10], c3[:, :, 0:10], c3[:, :, 10:20], op=OP.add)
                nc.vector.tensor_reduce(
                    s_px[:, g0 : g0 + gn], d3, axis=mybir.AxisListType.X, op=OP.add
                )
                if k == 1:
                    for c0 in range(0, KCOL, 512):
                        cn = min(512, KCOL - c0)
                        nc.tensor.matmul(
                            pen[:, c0 : c0 + cn],
                            ident96[:],
                            lgat[:, c0 : c0 + cn],
                            start=False,
                            stop=True,
                        )

            nc.vector.tensor_reduce(
                mstar[:],
                pen[:].rearrange("v (u k) -> v u k", k=K),
                axis=mybir.AxisListType.X,
                op=OP.min,
            )
            r_i = pp.tile([H, W], i32)
            nc.vector.tensor_scalar(
                r_i[:], mstar[:], 1.0 / STRIDE, -0.25, op0=OP.mult, op1=OP.add
            )
            r_f = pp.tile([H, W], f32)
            nc.vector.tensor_copy(r_f[:], r_i[:])
            lam = pp.tile([H, W], f32)
            nc.vector.scalar_tensor_tensor(
                lam[:], r_f[:], -STRIDE, mstar[:], op0=OP.mult, op1=OP.add
            )
            lmo = pp.tile([H, W], f32)
            nc.gpsimd.tensor_scalar(
                lmo[:], lam[:], 1.0, -OFF, op0=OP.mult, op1=OP.add
            )
            e_lam = pp.tile([H, W], f32)
            nc.scalar.activation(e_lam[:], lmo[:], AF.Exp)
            wgt = pp.tile([H, W], f32)
            nc.gpsimd.tensor_scalar(
                wgt[:], mstar[:], STRIDE * NBOX, 12.0, op0=OP.is_lt, op1=OP.mult
            )
            wq = pp.tile([H, W], f32)
            nc.gpsimd.tensor_scalar(wq[:], wgt[:], 1.0, 1.0, op0=OP.mult, op1=OP.add)

            lmo_px = pp.tile([128, HPX], f32)
            elam_px = pp.tile([128, HPX], f32)
            wq_px = pp.tile([128, HPX], f32)
            for dst, src in ((lmo_px, lmo), (elam_px, e_lam), (wq_px, wq)):
                nc.sync.dma_start(
                    dst[:], src[VC:H, :].rearrange("p (h t) -> p h t", h=2)
                )

            osum_ps = op_.tile([1, W + HPX], f32)

            def tail(s_ap, npart, width, lmo_g, elam_g, wq_g, ones, ocol):
                rs = pp.tile([npart, width], f32, tag=f"rs{npart}")
                nc.vector.reciprocal_approx_fast(rs[:], s_ap)
                lnb = pp.tile([npart, width], f32, tag=f"lnb{npart}")
                nc.vector.tensor_copy(lnb[:], s_ap.bitcast(i32))
                ln_s = pp.tile([npart, width], f32, tag=f"lns{npart}")
                nc.gpsimd.tensor_scalar(
                    ln_s[:],
                    lnb[:],
                    LN2 / (2.0**23),
                    (SLN - 127.0) * LN2,
                    op0=OP.mult,
                    op1=OP.add,
                )
                logp = pp.tile([npart, width], f32, tag=f"logp{npart}")
                nc.gpsimd.tensor_tensor(logp[:], lmo_g, ln_s[:], op=OP.subtract)
                p = pp.tile([npart, width], f32, tag=f"p{npart}")
                nc.vector.tensor_tensor(p[:], elam_g, rs[:], op=OP.mult)
                omm = pp.tile([npart, width], f32, tag=f"omm{npart}")
                nc.vector.tensor_scalar(
                    omm[:], p[:], -1.0, 1.0, op0=OP.mult, op1=OP.add
                )
                sq = pp.tile([npart, width], f32, tag=f"sq{npart}")
                nc.scalar.square(sq[:], omm[:])
                t1 = pp.tile([npart, width], f32, tag=f"t1{npart}")
                nc.vector.tensor_tensor(t1[:], sq[:], logp[:], op=OP.mult)
                wl = pp.tile([npart, width], bf16, tag=f"wl{npart}")
                nc.vector.tensor_tensor(wl[:], wq_g, t1[:], op=OP.mult)
                nc.tensor.matmul(
                    osum_ps[:, ocol : ocol + width],
                    ones,
                    wl[:],
                    start=True,
                    stop=True,
                )

            tail(
                s_ps[:], VC, W,
                lmo[0:VC, :], e_lam[0:VC, :], wq[0:VC, :],
                ones96[0:VC, :], 0,
            )
            tail(
                s_px[:], 128, HPX,
                lmo_px[:], elam_px[:], wq_px[:],
                ones128[:], W,
            )

            osum = pp.tile([1, 1], f32)
            nc.vector.tensor_reduce(
                osum[:], osum_ps[:], axis=mybir.AxisListType.X, op=OP.add
            )
            nc.sync.dma_start(out_d[:], osum[:])
            if dbg:
                nc.sync.dma_start(dbg_m[:], mstar[:])

    nc.compile()
    return nc


def _bin_of(depth):
    d = np.float32(depth)
    bin_size = np.float32(2.0 * (DEPTH_MAX - DEPTH_MIN) / (NUM_BINS * (1 + NUM_BINS)))
    idx = np.float32(-0.5) + np.float32(0.5) * np.sqrt(
        np.float32(1.0) + np.float32(8.0) * (d - np.float32(DEPTH_MIN)) / bin_size
    )
    bad = (idx < 0) | (idx > NUM_BINS) | ~np.isfinite(idx)
    idx = np.where(bad, np.float32(NUM_BINS), idx)
    return np.rint(idx).astype(np.int32)


def _host_prep(depth_logits, gt_boxes2d, num_gt_per_img, gt_center_depth):
    import ml_dtypes

    n = int(num_gt_per_img)
    boxes = np.asarray(gt_boxes2d, np.float32).reshape(B, n, 4)
    depths = np.asarray(gt_center_depth, np.float32).reshape(B, n)
    logits_f8 = np.asarray(depth_logits, np.float32).astype(ml_dtypes.float8_e4m3fn)

    consts8 = np.zeros((H, VC * VC + H), np.float32)
    for r in range(VC):
        consts8[:C, VC * r + r] = 1.0
    consts8[:, VC * VC :] = np.eye(H, dtype=np.float32)
    consts8 = consts8.astype(ml_dtypes.float8_e4m3fn)

    vs = np.arange(H, dtype=np.float32)

    in_maps = []
    for i in range(B):
        lg = logits_f8[i]
        lch = np.ascontiguousarray(lg[:, :VC, :].reshape(C, FC))
        blk = lg[:, VC:, :]
        t2 = blk.reshape(C, H - VC, 2, HPX).transpose(1, 2, 3, 0)
        lpx = np.ascontiguousarray(t2.reshape(128, PXCOL))

        bins = _bin_of(depths[i])
        order = np.argsort(bins, kind="stable")
        u1 = np.floor(boxes[i, order, 0]).astype(int)
        v1 = boxes[i, order, 1]
        u2 = np.ceil(boxes[i, order, 2]).astype(int)
        v2 = boxes[i, order, 3]
        cbins = bins[order]
        cand = np.full((W, K), NUM_BINS, np.int32)
        onehot = np.zeros((17, W * K), np.float32)
        cval = np.full((W * K), 2.0 * BIG, np.float32)
        for u in range(W):
            cov = [r for r in range(n) if u1[r] <= u < u2[r]]
            assert len(cov) <= K - 1, f"K too small: {len(cov)}"
            slots = [(16, NUM_BINS)] + [(r, cbins[r]) for r in cov]
            for s, (r, b) in enumerate(slots):
                cand[u, s] = b
                onehot[r, u * K + s] = 1.0
                cval[u * K + s] = BIG + STRIDE * r + OFF
        lg2 = lg.transpose(1, 2, 0)
        lgat = np.take_along_axis(
            lg2, np.broadcast_to(cand[None, :, :], (H, W, K)), axis=2
        ).reshape(H, KCOL)
        lgat = np.ascontiguousarray(lgat)

        rowm = (vs[None, :] >= np.floor(v1)[:, None]) & (
            vs[None, :] < np.ceil(v2)[:, None]
        )
        w18 = np.zeros((18, H), np.float32)
        w18[:16] = -BIG * rowm.astype(np.float32)
        w18[16] = -BIG
        w18[17] = 1.0
        bdc = np.concatenate([onehot, cval[None, :]], axis=0)
        bdcw = np.concatenate([bdc, w18], axis=1).astype(ml_dtypes.bfloat16)

        in_maps.append(
            {
                "lch": lch,
                "lpx": lpx,
                "lgat": lgat,
                "bdcw": bdcw,
                "consts8": consts8,
            }
        )
    return in_maps


def get_program():
    global _PROG
    if _PROG is None:
        _PROG = _build_program()
    return _PROG


def kernel(depth_logits, gt_boxes2d, num_gt_per_img, gt_center_depth, _trace=False):
    from concourse import bass_utils

    nc = get_program()
    in_maps = _host_prep(depth_logits, gt_boxes2d, num_gt_per_img, gt_center_depth)
    res = bass_utils.run_bass_kernel_spmd(
        nc, in_maps, core_ids=list(range(B)), trace=_trace
    )
    total = np.float64(0.0)
    for r in res.results:
        total += np.float64(r["out"].astype(np.float64).sum())
    loss = np.float32(-ALPHA * total / (B * H * W))
    if _trace:
        kernel._last_results = res
    return np.asarray(loss, dtype=np.float32)


# revision 16
# speedup vs baseline: 1.0396x; 1.0396x over previous
import sys

sys.path.insert(0, "/opt/trn_rl_repo")

import numpy as np

B, C, H, W = 8, 81, 96, 320
F = H * W
NBOX = 16
ALPHA = 0.25
DEPTH_MIN, DEPTH_MAX, NUM_BINS = 0.001, 60.0, 80

STRIDE = 32.0
OFF = 16.0
BIG = 1024.0
K = 7
KCOL = W * K

VC = 32
FC = VC * W
HPX = W // 2
PXCOL = (H - VC) * W * C // 128

NCH = 4
CHCOL = FC // NCH
PX_CHUNKS = ((0, 48), (48, 48), (96, 44), (140, 20))

CH_SPLIT = (1952, 608, 0)
PX_GSPLIT = {48: (29, 19, 0), 44: (27, 17, 0), 20: (13, 7, 0)}

A_SCH = 1024.0 * np.float64(np.log2(np.e))
B_SCH = 15360.0 - 60.0
LN2 = float(np.log(2.0))
SLN = 0.0435

_PROG = None


def _build_program():
    from concourse import bacc, tile, mybir

    f32 = mybir.dt.float32
    f16 = mybir.dt.float16
    bf16 = mybir.dt.bfloat16
    i16 = mybir.dt.int16
    i32 = mybir.dt.int32
    f8 = mybir.dt.float8e4
    AF = mybir.ActivationFunctionType
    OP = mybir.AluOpType

    nc = bacc.Bacc(
        "TRN2",
        target_bir_lowering=False,
        debug=False,
        enable_asserts=False,
    )

    lch_d = nc.dram_tensor("lch", [C, FC], f8, kind="ExternalInput")
    lpx_d = nc.dram_tensor("lpx", [128, PXCOL], f8, kind="ExternalInput")
    lgat_d = nc.dram_tensor("lgat", [H, KCOL], f8, kind="ExternalInput")
    bdcw_d = nc.dram_tensor("bdcw", [18, KCOL + H], bf16, kind="ExternalInput")
    consts8_d = nc.dram_tensor("consts8", [H, VC * VC + H], f8, kind="ExternalInput")
    out_d = nc.dram_tensor("out", [1, 1], f32, kind="ExternalOutput")

    import os

    dbg = os.environ.get("KERNEL_DEBUG") == "1"
    if dbg:
        dbg_m = nc.dram_tensor("dbg_m", [H, W], f32, kind="ExternalOutput")

    with tile.TileContext(nc) as tc:
        with (
            tc.tile_pool(name="persist", bufs=1) as pp,
            tc.tile_pool(name="tree", bufs=2) as tp,
            tc.tile_pool(name="spsum", bufs=1, space="PSUM") as sp,
            tc.tile_pool(name="ppsum", bufs=1, space="PSUM") as qp,
            tc.tile_pool(name="opsum", bufs=1, space="PSUM") as op_,
        ):
            lch = pp.tile([C, FC], f8)
            lpx = pp.tile([128, PXCOL], f8)
            ec = pp.tile([C, FC], f16)
            epx = pp.tile([128, PXCOL], f16)
            lgat = pp.tile([H, KCOL], f8)
            bdcw = pp.tile([18, KCOL + H], bf16)
            consts8 = pp.tile([H, VC * VC + H], f8)
            s_px = pp.tile([128, HPX], f32)
            ones96 = pp.tile([H, 1], bf16)
            ones128 = pp.tile([128, 1], bf16)
            nc.vector.memset(ones96[:], 1.0)
            nc.vector.memset(ones128[:], 1.0)
            diag32 = consts8[0:C, 0 : VC * VC]
            ident96 = consts8[:, VC * VC : VC * VC + H]

            def dma_lch(ring, k):
                ring.dma_start(
                    lch[:, k * CHCOL : (k + 1) * CHCOL],
                    lch_d[:, k * CHCOL : (k + 1) * CHCOL],
                )

            def dma_lpx(ring, k):
                g0, gn = PX_CHUNKS[k]
                ring.dma_start(
                    lpx[:, g0 * C : (g0 + gn) * C], lpx_d[:, g0 * C : (g0 + gn) * C]
                )

            dma_lch(nc.sync, 0)
            dma_lpx(nc.sync, 1)
            dma_lch(nc.sync, 3)
            dma_lpx(nc.scalar, 0)
            dma_lch(nc.scalar, 2)
            dma_lpx(nc.scalar, 3)
            nc.gpsimd.dma_start(consts8[:], consts8_d[:])
            nc.gpsimd.dma_start(bdcw[:], bdcw_d[:])
            nc.gpsimd.dma_start(lgat[:], lgat_d[:])
            dma_lch(nc.gpsimd, 1)
            dma_lpx(nc.gpsimd, 2)

            s_ps = sp.tile([VC, W], f32)
            pen = qp.tile([H, KCOL], f32)
            mstar = pp.tile([H, W], f32)

            w18 = bdcw[:, KCOL : KCOL + H]
            for c0 in range(0, KCOL, 512):
                cn = min(512, KCOL - c0)
                nc.tensor.matmul(
                    pen[:, c0 : c0 + cn],
                    w18,
                    bdcw[:, c0 : c0 + cn],
                    start=True,
                    stop=False,
                )

            def exp_split(dst, src, base, widths):
                a, p, d = widths
                c0 = base
                nc.scalar.activation(dst[:, c0 : c0 + a], src[:, c0 : c0 + a], AF.Exp)
                c0 += a
                nc.gpsimd.tensor_scalar(
                    dst[:, c0 : c0 + p].bitcast(i16),
                    src[:, c0 : c0 + p],
                    A_SCH,
                    B_SCH,
                    op0=OP.mult,
                    op1=OP.add,
                )
                c0 += p
                if d:
                    nc.vector.tensor_scalar(
                        dst[:, c0 : c0 + d].bitcast(i16),
                        src[:, c0 : c0 + d],
                        A_SCH,
                        B_SCH,
                        op0=OP.mult,
                        op1=OP.add,
                    )

            rows_per_chunk = VC // NCH
            for k in range(4):
                g0, gn = PX_CHUNKS[k]
                ga, gp, gd = PX_GSPLIT[gn]
                exp_split(epx, lpx, g0 * C, (ga * C, gp * C, gd * C))
                exp_split(ec, lch, k * CHCOL, CH_SPLIT)
                for r in range(rows_per_chunk):
                    v = k * rows_per_chunk + r
                    nc.tensor.matmul(
                        s_ps[:],
                        diag32[:, VC * v : VC * (v + 1)],
                        ec[:, v * W : (v + 1) * W],
                        start=(v == 0),
                        stop=(v == VC - 1),
                    )
                g3 = epx[:, g0 * C : (g0 + gn) * C].rearrange("p (g c) -> p g c", c=C)
                tb = tp.tile([128, gn * 40], f16, tag=f"tb{gn}")
                tc_ = tp.tile([128, gn * 20], f16, tag=f"tc{gn}")
                td = tp.tile([128, gn * 11], f16, tag=f"td{gn}")
                b3 = tb[:].rearrange("p (g c) -> p g c", c=40)
                c3 = tc_[:].rearrange("p (g c) -> p g c", c=20)
                d3 = td[:].rearrange("p (g c) -> p g c", c=11)
                TT = nc.vector.tensor_tensor
                TT(b3, g3[:, :, 0:40], g3[:, :, 40:80], op=OP.add)
                nc.vector.tensor_copy(d3[:, :, 10:11], g3[:, :, 80:81])
                TT(c3, b3[:, :, 0:20], b3[:, :, 20:40], op=OP.add)
                TT(d3[:, :, 0:10], c3[:, :, 0:10], c3[:, :, 10:20], op=OP.add)
                nc.vector.tensor_reduce(
                    s_px[:, g0 : g0 + gn], d3, axis=mybir.AxisListType.X, op=OP.add
                )
                if k == 1:
                    for c0 in range(0, KCOL, 512):
                        cn = min(512, KCOL - c0)
                        nc.tensor.matmul(
                            pen[:, c0 : c0 + cn],
                            ident96[:],
                            lgat[:, c0 : c0 + cn],
                            start=False,
                            stop=True,
                        )

            nc.vector.tensor_reduce(
                mstar[:],
                pen[:].rearrange("v (u k) -> v u k", k=K),
                axis=mybir.AxisListType.X,
                op=OP.min,
            )
            r_i = pp.tile([H, W], i32)
            nc.vector.tensor_scalar(
                r_i[:], mstar[:], 1.0 / STRIDE, -0.25, op0=OP.mult, op1=OP.add
            )
            r_f = pp.tile([H, W], f32)
            nc.vector.tensor_copy(r_f[:], r_i[:])
            lam = pp.tile([H, W], f32)
            nc.vector.scalar_tensor_tensor(
                lam[:], r_f[:], -STRIDE, mstar[:], op0=OP.mult, op1=OP.add
            )
            lmo = pp.tile([H, W], f32)
            nc.gpsimd.tensor_scalar(
                lmo[:], lam[:], 1.0, -OFF, op0=OP.mult, op1=OP.add
            )
            wgt = pp.tile([H, W], f32)
            nc.gpsimd.tensor_scalar(
                wgt[:], mstar[:], STRIDE * NBOX, 12.0, op0=OP.is_lt, op1=OP.mult
            )
            wq = pp.tile([H, W], f32)
            nc.gpsimd.tensor_scalar(wq[:], wgt[:], 1.0, 1.0, op0=OP.mult, op1=OP.add)

            lmo_px = pp.tile([128, HPX], f32)
            wq_px = pp.tile([128, HPX], f32)
            for dst, srct in ((lmo_px, lmo), (wq_px, wq)):
                nc.sync.dma_start(
                    dst[:], srct[VC:H, :].rearrange("p (h t) -> p h t", h=2)
                )

            osum_ps = op_.tile([1, W + HPX], f32)

            def tail(s_ap, npart, width, lmo_g, elam_g, wq_g, ones, ocol):
                lnb = pp.tile([npart, width], f32, tag=f"lnb{npart}")
                nc.vector.tensor_copy(lnb[:], s_ap.bitcast(i32))
                ln_s = pp.tile([npart, width], f32, tag=f"lns{npart}")
                nc.gpsimd.tensor_scalar(
                    ln_s[:],
                    lnb[:],
                    LN2 / (2.0**23),
                    (SLN - 127.0) * LN2,
                    op0=OP.mult,
                    op1=OP.add,
                )
                logp = pp.tile([npart, width], f32, tag=f"logp{npart}")
                nc.gpsimd.tensor_tensor(logp[:], lmo_g, ln_s[:], op=OP.subtract)
                p = pp.tile([npart, width], f32, tag=f"p{npart}")
                nc.scalar.activation(p[:], logp[:], AF.Exp)
                omm = pp.tile([npart, width], f32, tag=f"omm{npart}")
                nc.gpsimd.tensor_scalar(
                    omm[:], p[:], -1.0, 1.0, op0=OP.mult, op1=OP.add
                )
                sq = pp.tile([npart, width], f32, tag=f"sq{npart}")
                nc.scalar.square(sq[:], omm[:])
                t1 = pp.tile([npart, width], f32, tag=f"t1{npart}")
                nc.vector.tensor_tensor(t1[:], sq[:], logp[:], op=OP.mult)
                wl = pp.tile([npart, width], bf16, tag=f"wl{npart}")
                nc.vector.tensor_tensor(wl[:], wq_g, t1[:], op=OP.mult)
                nc.tensor.matmul(
                    osum_ps[:, ocol : ocol + width],
                    ones,
                    wl[:],
                    start=True,
                    stop=True,
                )

            tail(
                s_ps[:], VC, W,
                lmo[0:VC, :], None, wq[0:VC, :],
                ones96[0:VC, :], 0,
            )
            tail(
                s_px[:], 128, HPX,
                lmo_px[:], None, wq_px[:],
                ones128[:], W,
            )

            osum = pp.tile([1, 1], f32)
            nc.vector.tensor_reduce(
                osum[:], osum_ps[:], axis=mybir.AxisListType.X, op=OP.add
            )
            nc.sync.dma_start(out_d[:], osum[:])
            if dbg:
                nc.sync.dma_start(dbg_m[:], mstar[:])

    nc.compile()
    return nc


def _bin_of(depth):
    d = np.float32(depth)
    bin_size = np.float32(2.0 * (DEPTH_MAX - DEPTH_MIN) / (NUM_BINS * (1 + NUM_BINS)))
    idx = np.float32(-0.5) + np.float32(0.5) * np.sqrt(
        np.float32(1.0) + np.float32(8.0) * (d - np.float32(DEPTH_MIN)) / bin_size
    )
    bad = (idx < 0) | (idx > NUM_BINS) | ~np.isfinite(idx)
    idx = np.where(bad, np.float32(NUM_BINS), idx)
    return np.rint(idx).astype(np.int32)


def _host_prep(depth_logits, gt_boxes2d, num_gt_per_img, gt_center_depth):
    import ml_dtypes

    n = int(num_gt_per_img)
    boxes = np.asarray(gt_boxes2d, np.float32).reshape(B, n, 4)
    depths = np.asarray(gt_center_depth, np.float32).reshape(B, n)
    logits_f8 = np.asarray(depth_logits, np.float32).astype(ml_dtypes.float8_e4m3fn)

    consts8 = np.zeros((H, VC * VC + H), np.float32)
    for r in range(VC):
        consts8[:C, VC * r + r] = 1.0
    consts8[:, VC * VC :] = np.eye(H, dtype=np.float32)
    consts8 = consts8.astype(ml_dtypes.float8_e4m3fn)

    vs = np.arange(H, dtype=np.float32)

    in_maps = []
    for i in range(B):
        lg = logits_f8[i]
        lch = np.ascontiguousarray(lg[:, :VC, :].reshape(C, FC))
        blk = lg[:, VC:, :]
        t2 = blk.reshape(C, H - VC, 2, HPX).transpose(1, 2, 3, 0)
        lpx = np.ascontiguousarray(t2.reshape(128, PXCOL))

        bins = _bin_of(depths[i])
        order = np.argsort(bins, kind="stable")
        u1 = np.floor(boxes[i, order, 0]).astype(int)
        v1 = boxes[i, order, 1]
        u2 = np.ceil(boxes[i, order, 2]).astype(int)
        v2 = boxes[i, order, 3]
        cbins = bins[order]
        cand = np.full((W, K), NUM_BINS, np.int32)
        onehot = np.zeros((17, W * K), np.float32)
        cval = np.full((W * K), 2.0 * BIG, np.float32)
        for u in range(W):
            cov = [r for r in range(n) if u1[r] <= u < u2[r]]
            assert len(cov) <= K - 1, f"K too small: {len(cov)}"
            slots = [(16, NUM_BINS)] + [(r, cbins[r]) for r in cov]
            for s, (r, b) in enumerate(slots):
                cand[u, s] = b
                onehot[r, u * K + s] = 1.0
                cval[u * K + s] = BIG + STRIDE * r + OFF
        lg2 = lg.transpose(1, 2, 0)
        lgat = np.take_along_axis(
            lg2, np.broadcast_to(cand[None, :, :], (H, W, K)), axis=2
        ).reshape(H, KCOL)
        lgat = np.ascontiguousarray(lgat)

        rowm = (vs[None, :] >= np.floor(v1)[:, None]) & (
            vs[None, :] < np.ceil(v2)[:, None]
        )
        w18 = np.zeros((18, H), np.float32)
        w18[:16] = -BIG * rowm.astype(np.float32)
        w18[16] = -BIG
        w18[17] = 1.0
        bdc = np.concatenate([onehot, cval[None, :]], axis=0)
        bdcw = np.concatenate([bdc, w18], axis=1).astype(ml_dtypes.bfloat16)

        in_maps.append(
            {
                "lch": lch,
                "lpx": lpx,
                "lgat": lgat,
                "bdcw": bdcw,
                "consts8": consts8,
            }
        )
    return in_maps


def get_program():
    global _PROG
    if _PROG is None:
        _PROG = _build_program()
    return _PROG


def kernel(depth_logits, gt_boxes2d, num_gt_per_img, gt_center_depth, _trace=False):
    from concourse import bass_utils

    nc = get_program()
    in_maps = _host_prep(depth_logits, gt_boxes2d, num_gt_per_img, gt_center_depth)
    res = bass_utils.run_bass_kernel_spmd(
        nc, in_maps, core_ids=list(range(B)), trace=_trace
    )
    total = np.float64(0.0)
    for r in res.results:
        total += np.float64(r["out"].astype(np.float64).sum())
    loss = np.float32(-ALPHA * total / (B * H * W))
    if _trace:
        kernel._last_results = res
    return np.asarray(loss, dtype=np.float32)
